# revision 20
# baseline (speedup 1.0000x reference)
"""Trainium2 Bass kernel for nn_CIN_81544249082266 (CIN / xDeepFM cross network).

Pure data parallel over 8 NeuronCores: each core processes 1024 of the 8192
batch rows; filters and output weights are replicated. No cross-device
communication (the host concatenates the per-core [1024] score vectors).

Math (per sample b, embedding dim d in [0,16), fields F0=39):
  layer k: z[(i,j), (b,d)] = x0[i,(b,d)] * h_k[j,(b,d)];  curr = relu(F_k^T z)
  h_{k+1} = curr rows [0:64), direct_k = remaining rows
  score[b] = sum_{m,d} direct[m,(b,d)] * (1 + w_nn[m]) + b_nn

On-chip layout: everything lives transposed ("rows" r=(b*16+d) on the free
axis), so each layer's PSUM output [128 l-partitions, r] is directly the next
layer's h operand — no transposes anywhere. The replicated-x operand A
(row i broadcast across 64 partitions) is materialized by broadcast DMA;
the tiled-h operand is materialized by writing the relu output twice.
"""

import numpy as np
from contextlib import ExitStack

import concourse.bass as bass
import concourse.tile as tile
from concourse import bacc, mybir
from concourse.bass_utils import run_bass_kernel_spmd

F0 = 39
D = 16
B = 8192
NCORES = 8
BC = B // NCORES            # 1024 samples per core
N = BC * D                  # 16384 r-columns per core
CH = 1024                   # chunk of r processed per inner iteration
NCHUNK = N // CH            # 16
NBLK = 20                   # ceil(40*64/128) c-blocks per layer (i padded to 40)
NBLK_L0 = 19                # block 19 (i=38,39) is all-zero after the triu mask
FP16 = mybir.dt.float16
FP32 = mybir.dt.float32

_BUILT = None


def _build_program():
    """Build + compile the 8-core SPMD Bass program once per process."""
    nc = bacc.Bacc(
        "TRN2",
        target_bir_lowering=False,
        debug=False,
        num_devices=NCORES,
    )

    x2_d = nc.dram_tensor("x2", [64, N], FP16, kind="ExternalInput").ap()
    f_d = [
        nc.dram_tensor(f"f{k}", [128, NBLK * 128], FP16, kind="ExternalInput").ap()
        for k in range(3)
    ]
    wv_d = nc.dram_tensor("wv", [128, 3], FP16, kind="ExternalInput").ap()
    bias_d = nc.dram_tensor("bias", [1, 1], FP32, kind="ExternalInput").ap()
    out_d = nc.dram_tensor("out", [1, BC], FP32, kind="ExternalOutput").ap()

    relu = mybir.ActivationFunctionType.Relu
    ndma = [0]

    with tile.TileContext(nc) as tc, ExitStack() as ctx:
        const = ctx.enter_context(tc.tile_pool(name="const", bufs=1))
        pool_a = ctx.enter_context(tc.tile_pool(name="a", bufs=64))
        pool_x = ctx.enter_context(tc.tile_pool(name="x", bufs=5))
        pool_z = ctx.enter_context(tc.tile_pool(name="z", bufs=8))
        pool_h = ctx.enter_context(tc.tile_pool(name="h", bufs=3))
        pool_dt = ctx.enter_context(tc.tile_pool(name="dt", bufs=3))
        pool_r2 = ctx.enter_context(tc.tile_pool(name="r2", bufs=3))
        ps_curr = ctx.enter_context(tc.tile_pool(name="pcur", bufs=3, space="PSUM"))
        ps_s = ctx.enter_context(tc.tile_pool(name="ps", bufs=2, space="PSUM"))

        # --- resident constants ---
        fsb = []
        for k in range(3):
            f = const.tile([128, NBLK * 128], FP16, tag=f"f{k}")
            nc.gpsimd.dma_start(f[:, : NBLK * 64], f_d[k][:, : NBLK * 64])
            nc.gpsimd.dma_start(f[:, NBLK * 64 :], f_d[k][:, NBLK * 64 :])
            fsb.append(f)
        wv = const.tile([128, 3], FP16)
        nc.sync.dma_start(wv[:], wv_d[:])
        bias = const.tile([1, 1], FP32)
        nc.sync.dma_start(bias[:], bias_d[:])
        scores = const.tile([1, BC], FP32)

        def load_chunk(c):
            """A-broadcast DMAs + x-chunk for chunk c, straight from HBM.
            The x-chunk and low-k tiles are needed first, so they go on the
            gpsimd SW DGE (sprays across all 16 SDMA engines); the sync/
            scalar HW queues (one shared slow 2-engine SDMA pair) only get
            the last two k's of each chunk."""
            sl = slice(c * CH, (c + 1) * CH)
            xc = pool_x.tile([128, CH], FP16, tag="xc", name=f"xc_{c}")
            nc.sync.dma_start(xc[0:64, :], x2_d[:, sl])
            nc.scalar.dma_start(xc[64:128, :], x2_d[:, sl])
            a_tiles = []
            for k in range(NBLK):
                a = pool_a.tile([128, CH], FP16, tag="a", name=f"a_{c}_{k}")
                src = x2_d[2 * k : 2 * k + 2, None, sl].to_broadcast([2, 64, CH])
                if k == 18:
                    nc.scalar.dma_start(a[:], src)
                elif k == 19:
                    nc.sync.dma_start(a[:], src)
                else:
                    nc.gpsimd.dma_start(a[:], src, single_packet=True)
                a_tiles.append(a)
            return a_tiles, xc

        def layer_pass(c, layer, a_tiles, b_op, fw):
            """One 1024-wide k-sweep: z = a*b on DVE feeding accumulating
            matmuls; returns the PSUM tile."""
            nblk = NBLK_L0 if layer == 0 else NBLK
            cur = ps_curr.tile([128, CH], FP32, tag="cur", name=f"cur_{c}_{layer}")
            for k in range(nblk):
                z = pool_z.tile([128, CH], FP16, tag="z", name=f"z_{c}_{layer}_{k}")
                nc.vector.tensor_tensor(
                    out=z[:], in0=a_tiles[k][:], in1=b_op[:],
                    op=mybir.AluOpType.mult,
                )
                for sgn in range(2):
                    ssl = slice(sgn * 512, (sgn + 1) * 512)
                    nc.tensor.matmul(
                        cur[:, ssl],
                        lhsT=fw[:, k * 128 : (k + 1) * 128],
                        rhs=z[:, ssl],
                        start=(k == 0),
                        stop=(k == nblk - 1),
                    )
            return cur

        def score_mms(sab, layer, rhs_t, rhs_k):
            for sgn in range(2):
                ssl = slice(sgn * 512, (sgn + 1) * 512)
                nc.tensor.matmul(
                    sab[32 * sgn : 32 * sgn + 1, :],
                    lhsT=wv[0:rhs_k, layer : layer + 1],
                    rhs=rhs_t[0:rhs_k, ssl],
                    start=(layer == 0), stop=(layer == 2),
                    tile_position=(0, 32 * sgn),
                )

        # Software-pipelined schedule: L0 runs one chunk ahead so the PE
        # always has an independent pass to stream while the previous pass
        # drains through ACT (relu) into the next layer's operand.
        def do_l0(c, a_tiles, xc):
            cur = layer_pass(c, 0, a_tiles, xc, fsb[0])
            h_t = pool_h.tile([128, CH], FP16, tag="h", name=f"h_{c}")
            d_t = pool_dt.tile([64, CH], FP16, tag="d", name=f"d_{c}")
            nc.scalar.activation(h_t[0:64, :], cur[0:64, :], relu)
            nc.scalar.activation(h_t[64:128, :], cur[0:64, :], relu)
            nc.scalar.activation(d_t[:], cur[64:128, :], relu)
            sab = ps_s.tile([33, 512], FP32, tag="sab", name=f"sab_{c}")
            score_mms(sab, 0, d_t, 64)
            return h_t, sab

        chunks = {}
        for cc in range(3):
            chunks[cc] = load_chunk(cc)
        h_t, sab = do_l0(0, chunks[0][0], chunks[0][1])
        state = (h_t, sab)

        def emit_reduces(t, sab_t):
            # sum over d (innermost 16): each half-score row -> 32 b-scores
            for sgn in range(2):
                off = t * (CH // D) + sgn * 32
                nc.vector.tensor_reduce(
                    out=scores[0:1, off : off + 32],
                    in_=sab_t[32 * sgn : 32 * sgn + 1, :].rearrange(
                        "p (g x) -> p g x", x=D
                    ),
                    axis=mybir.AxisListType.X,
                    op=mybir.AluOpType.add,
                )

        pending_reduce = None
        for t in range(NCHUNK):
            if t + 3 < NCHUNK:
                chunks[t + 3] = load_chunk(t + 3)
            a_tiles, _ = chunks[t]
            h1, sab = state
            # L1(t)
            cur1 = layer_pass(t, 1, a_tiles, h1, fsb[1])
            # previous chunk's L2 score MMs + reduce, emitted here (r2 and
            # sab have been ready since mid-previous iteration) so neither
            # the PE nor the DVE ever stalls on them
            if pending_reduce is not None:
                pt, psab, pr2 = pending_reduce
                score_mms(psab, 2, pr2, 128)
                emit_reduces(pt, psab)
            h2 = pool_h.tile([128, CH], FP16, tag="h", name=f"h2_{t}")
            d1 = pool_dt.tile([64, CH], FP16, tag="d", name=f"d1_{t}")
            nc.scalar.activation(h2[0:64, :], cur1[0:64, :], relu)
            nc.scalar.activation(h2[64:128, :], cur1[0:64, :], relu)
            nc.scalar.activation(d1[:], cur1[64:128, :], relu)
            # L0(t+1) streams while L1(t) drains through ACT
            if t + 1 < NCHUNK:
                state = do_l0(t + 1, chunks[t + 1][0], chunks[t + 1][1])
            score_mms(sab, 1, d1, 64)
            # L2(t)
            cur2 = layer_pass(t, 2, a_tiles, h2, fsb[2])
            r2 = pool_r2.tile([128, CH], FP16, tag="r2", name=f"r2_{t}")
            nc.scalar.activation(r2[:], cur2[:], relu)
            del chunks[t]
            pending_reduce = (t, sab, r2)

        pt, psab, pr2 = pending_reduce
        score_mms(psab, 2, pr2, 128)
        emit_reduces(pt, psab)
        nc.vector.tensor_scalar_add(scores[:], scores[:], bias[0:1, 0:1])
        nc.sync.dma_start(out_d[:], scores[:])

    nc.compile()
    return nc


def _prep_inputs(nn_input, f0, f1, f2, w_nn, b_nn):
    """Host-side preprocessing into the kernel's layouts."""
    nn_input = np.asarray(nn_input, dtype=np.float32)
    f0 = np.asarray(f0, dtype=np.float32)
    f1 = np.asarray(f1, dtype=np.float32)
    f2 = np.asarray(f2, dtype=np.float32)
    w_nn = np.asarray(w_nn, dtype=np.float32).reshape(-1)
    b_nn = np.asarray(b_nn, dtype=np.float32).reshape(-1)

    # filters -> [40*64, 128] (i-major, j in [0,64)), then lhsT blocks
    def pack(fp):  # fp: [2560, 128] -> [128, NBLK*128]
        blocks = fp.reshape(NBLK, 128, 128)
        return np.ascontiguousarray(
            blocks.transpose(1, 0, 2).reshape(128, NBLK * 128)
        ).astype(np.float16)

    f0p = np.zeros((40, 64, 128), np.float32)
    f0r = f0.reshape(F0, F0, 128)
    iu, ju = np.triu_indices(F0, k=1)
    f0p[iu, ju] = 2.0 * f0r[iu, ju]
    f0p = pack(f0p.reshape(2560, 128))

    def padf(f):  # [39*64, 128] -> [2560, 128]
        out = np.zeros((2560, 128), np.float32)
        out[: F0 * 64] = f
        return pack(out)

    f1p, f2p = padf(f1), padf(f2)

    wv = np.zeros((128, 3), np.float32)
    wv[0:64, 0] = 1.0 + w_nn[0:64]    # layer-0 direct weights (res + w_nn)
    wv[0:64, 1] = 1.0 + w_nn[64:128]  # layer-1 direct weights
    wv[:, 2] = 1.0 + w_nn[128:256]    # layer-2 direct weights
    wv = wv.astype(np.float16)
    bias = b_nn.reshape(1, 1).astype(np.float32)

    # x2 per core: [64, N] fp16 with rows 0..38 = x^T, rest zero
    x0 = nn_input.reshape(B, F0, D)
    in_maps = []
    for cidx in range(NCORES):
        xc = x0[cidx * BC : (cidx + 1) * BC]            # [BC, 39, 16]
        xt = xc.transpose(1, 0, 2).reshape(F0, N)        # [39, (b,d)]
        x2h = np.zeros((64, N), np.float16)
        x2h[:F0] = xt.astype(np.float16)
        in_maps.append(
            {"x2": x2h, "f0": f0p, "f1": f1p, "f2": f2p, "wv": wv, "bias": bias}
        )
    return in_maps


def _run(inputs, trace=False, trace_kwargs=None):
    global _BUILT
    if _BUILT is None:
        _BUILT = _build_program()
    nc = _BUILT
    in_maps = _prep_inputs(**inputs)
    res = run_bass_kernel_spmd(
        nc,
        in_maps,
        core_ids=list(range(NCORES)),
        trace=trace,
        **(trace_kwargs or {}),
    )
    out = np.concatenate(
        [res.results[c]["out"].reshape(BC) for c in range(NCORES)]
    )
    return out.reshape(B, 1).astype(np.float32), res


def kernel(**inputs):
    out, _ = _run(inputs)
    return out
